# revision 13
# baseline (speedup 1.0000x reference)
"""Trainium2 Bass kernel: doc-conditioned embedding lookup + scoring.

Per sample b:
    x[b]        = sum_c ( D[doc_ids[b], context_ids[b,c]] + W[context_ids[b,c]] )
    result[b,t] = dot(x[b], O[:, target_noise_ids[b,t]])

Distribution over 8 NeuronCores: samples are stable-sorted by doc_id and split
into 8 equal chunks of 512, so every core does identical work (perfect
balance).  Core c receives only the K_DOCS-row window of D that its chunk's
doc_ids span (doc/expert sharding per the hint).  W and O^T are concatenated
into one small 16000-row table, replicated per core; each 128-sample tile's
24 per-sample vectors from it (8 W rows + 16 O columns) are fetched with a
single int16 dma_gather.  The per-(sample, ctx) D rows (window too large for
int16 gather indices) are fetched with 8 singleton-offset indirect DMAs per
tile, which is the layout hardware supports.  Results are scattered back
through the sort permutation on the host.
"""

import numpy as np

try:  # persistent XLA/NEFF compile cache: makes repeat runs fast
    import jax

    jax.config.update("jax_compilation_cache_dir", "/tmp/jax_cache")
    jax.config.update("jax_persistent_cache_min_compile_time_secs", 0.0)
    jax.config.update("jax_persistent_cache_min_entry_size_bytes", 0)
except Exception:
    pass

import concourse.bass as bass
import concourse.mybir as mybir
import concourse.tile as tile
from concourse.bacc import Bacc
from concourse.bass_utils import run_bass_kernel_spmd

N_CORES = 8
BATCH, N_CTX, N_TGT = 4096, 8, 16
NUM_DOCS, NUM_WORDS, VEC_DIM = 500, 8000, 128
PER_CORE = BATCH // N_CORES  # 512
P = 128
K_DOCS = 80  # docs shipped per core; covers any 512-sample chunk's doc span

_nc_cache: dict = {}


def build_nc(
    per_core=PER_CORE,
    n_ctx=N_CTX,
    n_tgt=N_TGT,
    vec_dim=VEC_DIM,
    num_words=NUM_WORDS,
    k_docs=K_DOCS,
    reps=1,
    fast=False,
):
    """Build the per-core Bass program (SPMD: same program on all cores).

    fast=True: spread SWDGE work over two descriptor queues (dma_gathers on
    queue 1, indirect DMAs on queue 0) and issue plain index/output DMAs on
    the HWDGE sync engine instead of gpsimd, to relieve Pool-engine issue
    pressure.
    """
    n_tiles = per_core // P
    assert per_core % P == 0
    g = n_ctx + n_tgt  # vectors per sample from the W/OT table
    n_wog = g * P  # dma_gather indices per tile
    wog_cols = n_wog // 16  # wrapped int16 index columns per tile

    nc = Bacc(num_swdge_queues=2 if fast else 1)
    plain_eng = nc.sync if fast else nc.gpsimd
    gather_q = 1 if fast else 0
    dslab = nc.declare_dram_parameter(
        "dslab", [k_docs * num_words, vec_dim], mybir.dt.float32, isOutput=False
    )
    wot = nc.declare_dram_parameter(
        "wot", [2 * num_words, vec_dim], mybir.dt.float32, isOutput=False
    )
    didx = nc.declare_dram_parameter(
        "didx", [per_core, n_ctx], mybir.dt.int32, isOutput=False
    )
    wogidx = nc.declare_dram_parameter(
        "wogidx", [P, n_tiles * wog_cols], mybir.dt.int16, isOutput=False
    )
    out = nc.declare_dram_parameter(
        "out", [per_core, n_tgt], mybir.dt.float32, isOutput=True
    )

    with tile.TileContext(nc) as tc:
        with (
            tc.tile_pool(name="idx", bufs=1) as idx_pool,
            tc.tile_pool(name="gather", bufs=3) as gpool,
            tc.tile_pool(name="small", bufs=3) as spool,
        ):
            # All index tiles upfront. didx row (t*128 + p) -> partition p, col t*n_ctx.
            didx_all = idx_pool.tile([P, n_tiles * n_ctx], mybir.dt.int32, tag="didx")
            plain_eng.dma_start(
                out=didx_all[:],
                in_=didx[:, :].rearrange("(t p) j -> p t j", p=P),
            )
            wogidx_all = idx_pool.tile(
                [P, n_tiles * wog_cols], mybir.dt.int16, tag="wogidx"
            )
            plain_eng.dma_start(out=wogidx_all[:], in_=wogidx[:, :])
            scores_all = idx_pool.tile(
                [P, n_tiles * n_tgt], mybir.dt.float32, tag="scores"
            )

            # hardware caps one dynamic DMA at 1024 descriptors (16KB SWDGE
            # descriptor carveout), so split each tile's gather into 1024-idx
            # sub-gathers of 8 blocks each
            sub = min(g, 1024 // P)  # blocks per sub-gather
            assert g % sub == 0
            n_sub = g // sub
            sub_cols = sub * P // 16
            for i in [t for _ in range(reps) for t in range(n_tiles)]:
                # wog[p, u, :]: u<n_ctx -> W[ctx[s,u]], else OT[tgt[s,u-n_ctx]]
                wog = gpool.tile([P, g * vec_dim], mybir.dt.float32, tag="wog")
                for k in range(n_sub):
                    nc.gpsimd.dma_gather(
                        out_ap=wog[:, k * sub * vec_dim : (k + 1) * sub * vec_dim]
                        .rearrange("p (b e) -> p b e", e=vec_dim),
                        in_ap=wot[:],
                        idxs_ap=wogidx_all[
                            :, i * wog_cols + k * sub_cols : i * wog_cols + (k + 1) * sub_cols
                        ],
                        num_idxs=sub * P,
                        num_idxs_reg=sub * P,
                        elem_size=vec_dim,
                        queue_num=gather_q,
                    )
                # dtile[p, c, :] = D[doc[s], ctx[s, c]] (window-local rows)
                dtile = gpool.tile([P, n_ctx * vec_dim], mybir.dt.float32, tag="dtile")
                for c in range(n_ctx):
                    nc.gpsimd.indirect_dma_start(
                        out=dtile[:, bass.ts(c, vec_dim)],
                        out_offset=None,
                        in_=dslab[:],
                        in_offset=bass.IndirectOffsetOnAxis(
                            ap=didx_all[:, i * n_ctx + c : i * n_ctx + c + 1], axis=0
                        ),
                    )

                # x[s, v] = sum_c dtile[s, c, v] + sum_c wog[s, c, v]
                xd = spool.tile([P, vec_dim], mybir.dt.float32, tag="xd")
                nc.vector.reduce_sum(
                    out=xd[:],
                    in_=dtile[:].rearrange("p (c v) -> p v c", c=n_ctx),
                    axis=mybir.AxisListType.X,
                )
                xw = spool.tile([P, vec_dim], mybir.dt.float32, tag="xw")
                nc.vector.reduce_sum(
                    out=xw[:],
                    in_=wog[:, : n_ctx * vec_dim].rearrange(
                        "p (c v) -> p v c", c=n_ctx
                    ),
                    axis=mybir.AxisListType.X,
                )
                x = spool.tile([P, vec_dim], mybir.dt.float32, tag="x")
                nc.vector.tensor_add(x[:], xd[:], xw[:])

                # prod[s, t, v] = og[s, t, v] * x[s, v]
                prod = gpool.tile([P, n_tgt * vec_dim], mybir.dt.float32, tag="prod")
                og_view = wog[:, n_ctx * vec_dim :].rearrange(
                    "p (t v) -> p t v", t=n_tgt
                )
                nc.vector.tensor_mul(
                    out=prod[:].rearrange("p (t v) -> p t v", t=n_tgt),
                    in0=og_view,
                    in1=x[:].unsqueeze(1).to_broadcast([P, n_tgt, vec_dim]),
                )

                # scores[s, t] = sum_v prod[s, t, v]
                nc.vector.reduce_sum(
                    out=scores_all[:, bass.ts(i, n_tgt)],
                    in_=prod[:].rearrange("p (t v) -> p t v", t=n_tgt),
                    axis=mybir.AxisListType.X,
                )

            plain_eng.dma_start(
                out=out[:, :].rearrange("(t p) j -> p t j", p=P),
                in_=scores_all[:],
            )
    nc.finalize()
    return nc


def _get_nc(k_docs):
    if k_docs not in _nc_cache:
        _nc_cache[k_docs] = build_nc(k_docs=k_docs)
    return _nc_cache[k_docs]


def _wrap_wog_indices(ctx, tgt, num_words, n_tiles):
    """Build the [128, n_tiles*cols] int16 wrapped index layout for dma_gather.

    Per tile: index j (0..g*128) -> block b=j//128 (vector slot), partition
    p=j%128 (sample).  Wrapped storage: j at [j%16, j//16], replicated across
    the 8 groups of 16 partitions.
    """
    n_ctx, n_tgt = ctx.shape[1], tgt.shape[1]
    g = n_ctx + n_tgt
    per_tile = []
    for t in range(n_tiles):
        sl = slice(t * P, (t + 1) * P)
        vals = np.concatenate(
            [ctx[sl].T, num_words + tgt[sl].T], axis=0
        )  # [g, 128]; vals[b, p] = index for j = b*128 + p
        unwrapped = vals.reshape(g * P)  # j-major
        wrapped = unwrapped.reshape(-1, 16).T  # [16, cols]
        per_tile.append(np.tile(wrapped, (8, 1)))  # [128, cols]
    return np.ascontiguousarray(np.concatenate(per_tile, axis=1).astype(np.int16))


def make_in_maps(context_ids, doc_ids, target_noise_ids, D, W, O, k_docs=K_DOCS):
    """Host-side routing/sharding. Returns (in_maps, chunks, k_docs)."""
    ctx = np.asarray(context_ids).astype(np.int64)
    doc = np.asarray(doc_ids).astype(np.int64)
    tgt = np.asarray(target_noise_ids).astype(np.int64)
    D = np.ascontiguousarray(np.asarray(D, dtype=np.float32))
    W = np.ascontiguousarray(np.asarray(W, dtype=np.float32))
    ot = np.asarray(O, dtype=np.float32).T

    num_docs, num_words, vec_dim = D.shape
    wot = np.ascontiguousarray(np.concatenate([W, ot], axis=0))
    perm = np.argsort(doc, kind="stable")
    chunks = perm.reshape(N_CORES, -1)
    n_tiles = chunks.shape[1] // P

    los = []
    for c in range(N_CORES):
        d = doc[chunks[c]]
        lo, hi = int(d.min()), int(d.max())
        span = hi - lo + 1
        if span > k_docs:
            k_docs = span  # pathological doc distribution; wider window
        los.append(lo)
    k_docs = min(k_docs, num_docs)

    D2 = D.reshape(num_docs * num_words, vec_dim)
    in_maps = []
    for c in range(N_CORES):
        lo = min(max(los[c], 0), num_docs - k_docs)
        sl = chunks[c]
        dslab = D2[lo * num_words : (lo + k_docs) * num_words]
        didx = ((doc[sl] - lo)[:, None] * num_words + ctx[sl]).astype(np.int32)
        in_maps.append(
            {
                "dslab": dslab,
                "wot": wot,
                "didx": didx,
                "wogidx": _wrap_wog_indices(ctx[sl], tgt[sl], num_words, n_tiles),
            }
        )
    return in_maps, chunks, k_docs


def run(inputs: dict, trace: bool = False, trace_cores=None):
    """Run on hardware; returns (full_output, BassKernelResults)."""
    in_maps, chunks, k_docs = make_in_maps(**inputs)
    nc = _get_nc(k_docs)
    res = run_bass_kernel_spmd(
        nc, in_maps, list(range(N_CORES)), trace=trace, trace_cores=trace_cores
    )
    out = np.empty((chunks.size, N_TGT), np.float32)
    for c in range(N_CORES):
        out[chunks[c]] = res.results[c]["out"]
    return out, res


def kernel(context_ids, doc_ids, target_noise_ids, D, W, O):
    out, _ = run(
        {
            "context_ids": context_ids,
            "doc_ids": doc_ids,
            "target_noise_ids": target_noise_ids,
            "D": D,
            "W": W,
            "O": O,
        }
    )
    return out
